# revision 1
# baseline (speedup 1.0000x reference)
"""Trainium2 kernel for nn_AttentionModel_PCA (embedding_lookup).

Math: with sf = softmax(Q^T K) per head,
  G[i,m,a] = sum_h sum_j sf[h,i,j] * V[h,a,Z2[j,m]]
           = sum_{(j,c)} T[(j,c),(i,a)] * E[(j,c),m]
where T[(j,c),(i,a)] = sum_h sf[h,i,j] V[h,a,c]  (tiny H=8 contraction)
and E is the one-hot expansion of Z2. The (5376 x 5376) @ (5376 x M)
GEMM producing G is the dominant compute/memory cost and runs on the 8
NeuronCores with M sharded (512 samples per core, per the data-parallel
hint). Host does the cheap prep (softmax, T, E) and the small tail
(take_along_axis, logsumexp, weighted sums, regularizer).
"""

import sys

import numpy as np

for _p in ("/opt/trn_rl_repo",):
    if _p not in sys.path:
        sys.path.append(_p)

H, d, N1, N2, q1, q2, M = 8, 64, 256, 256, 21, 21, 4096
NCORES = 8
MS = M // NCORES          # 512 samples per core
KDIM = N2 * q2            # 5376 contraction (j,c)
PDIM = N1 * q1            # 5376 output rows (i,a)
KT = KDIM // 128          # 42 contraction tiles
PT = PDIM // 128          # 42 output-row tiles
PGROUP = 3                # PSUM banks per group (must divide PT)
LAMBD = 0.001

_PROGRAM = None


def _build_program():
    """Raw bass pipeline (no Tile): explicit standalone wait_ge + then_inc,
    mirroring the known-good SPMD test patterns.

    sync  : E load, then 42*11 slab loads (4-slot ring, gated by PE use)
    tensor: 1764 accumulating matmuls, 2 PSUM bank-sets ping-pong
    vector: PSUM -> SBUF copies (4-slot out ring, gated by stores)
    scalar: SBUF -> G stores (HWDGE; gpsimd SWDGE deadlocks vs concurrent DVE)
    """
    import concourse.bass as bass
    import concourse.mybir as mybir

    nc = bass.Bass()
    f32 = mybir.dt.float32
    Tt = nc.declare_dram_parameter("Tt", [KDIM, PDIM], f32, isOutput=False)
    E = nc.declare_dram_parameter("E", [KDIM, MS], f32, isOutput=False)
    G = nc.declare_dram_parameter("G", [PDIM, MS], f32, isOutput=True)

    Tt_v = Tt.rearrange("(t p) c -> t p c", p=128)
    E_v = E.rearrange("(t p) m -> p t m", p=128)

    NG = PT // PGROUP        # 42/4 (PGROUP must divide PT)
    assert NG * PGROUP == PT
    NS = 4                   # slab ring slots
    NOT = 4                  # output ring slots

    with (
        nc.sbuf_tensor([128, KT, MS], f32) as E_sb,
        nc.sbuf_tensor([128, NS, PGROUP * 128], f32) as slab,
        nc.sbuf_tensor([128, NOT, MS], f32) as ot,
        nc.psum_tensor([128, 2 * PGROUP * MS], f32) as acc,
        nc.semaphore("e_sem") as e_sem,
        nc.semaphore("dma_sem") as dma_sem,
        nc.semaphore("pe_cnt") as pe_cnt,      # slabs fully consumed by PE
        nc.semaphore("cp_sem") as cp_sem,      # PSUM->SBUF copies done
        nc.semaphore("st_sem") as st_sem,      # G stores done
        nc.Block() as block,
    ):

        @block.sync
        def _(sync):
            sync.dma_start(E_sb[:], E_v).then_inc(e_sem, 16)
            for n in range(NG * KT):
                g, t = divmod(n, KT)
                if n >= NS:
                    sync.wait_ge(pe_cnt, n - NS + 1)
                sync.dma_start(
                    slab[:, n % NS, :],
                    Tt_v[t, :, g * PGROUP * 128:(g + 1) * PGROUP * 128],
                ).then_inc(dma_sem, 16)

        @block.tensor
        def _(tensor):
            tensor.wait_ge(e_sem, 16)
            for g in range(NG):
                half = (g % 2) * PGROUP * MS
                if g >= 2:
                    # banks reused from group g-2: wait for its copies
                    tensor.wait_ge(cp_sem, (g - 1) * PGROUP)
                for t in range(KT):
                    n = g * KT + t
                    tensor.wait_ge(dma_sem, 16 * (n + 1))
                    for j in range(PGROUP):
                        mm = nc.tensor.matmul(
                            acc[:, half + j * MS:half + (j + 1) * MS],
                            slab[:, n % NS, j * 128:(j + 1) * 128],
                            E_sb[:, t, :],
                            start=(t == 0),
                            stop=(t == KT - 1),
                        )
                        if j == PGROUP - 1:
                            mm.then_inc(pe_cnt, 1)

        @block.vector
        def _(vector):
            for g in range(NG):
                half = (g % 2) * PGROUP * MS
                vector.wait_ge(pe_cnt, (g + 1) * KT)
                for j in range(PGROUP):
                    k = g * PGROUP + j
                    if k >= NOT:
                        vector.wait_ge(st_sem, 16 * (k - NOT + 1))
                    nc.vector.tensor_copy(
                        ot[:, k % NOT, :],
                        acc[:, half + j * MS:half + (j + 1) * MS],
                    ).then_inc(cp_sem, 1)

        @block.scalar
        def _(scalar):
            for k in range(NG * PGROUP):
                scalar.wait_ge(cp_sem, k + 1)
                scalar.dma_start(
                    G[k * 128:(k + 1) * 128, :], ot[:, k % NOT, :]
                ).then_inc(st_sem, 16)

    return nc


def host_prep(Q, K, V, Z2):
    """softmax, T (transposed for the PE), one-hot E. All cheap."""
    e = np.einsum("hdi,hdj->hij", Q, K, optimize=True)
    e -= e.max(axis=2, keepdims=True)
    np.exp(e, out=e)
    sf = e / e.sum(axis=2, keepdims=True)
    Tt = np.einsum("hij,hac->jcia", sf, V, optimize=True).reshape(KDIM, PDIM)
    Mloc = Z2.shape[1]
    Eoh = np.zeros((KDIM, Mloc), np.float32)
    rows = (np.arange(N2, dtype=np.int64)[:, None] * q2 + Z2.astype(np.int64))
    Eoh[rows, np.arange(Mloc, dtype=np.int64)[None, :]] = 1.0
    return sf, np.ascontiguousarray(Tt, np.float32), Eoh


def host_tail(G, sf, V, Z1, weights):
    """take_along_axis + logsumexp + loss + regularizer on (N1, M, q1) G."""
    Z1i = Z1.astype(np.int64)
    mat_ene_sum = np.take_along_axis(G, Z1i[:, :, None], axis=2)[..., 0].sum(axis=0)

    Gm = G.max(axis=0)                                   # (M, q1)
    L = np.log(np.exp(G - Gm).sum(axis=0)) + Gm          # (M, q1)
    mx = np.maximum(L.max(axis=1), 0.0)
    logZ = np.log(np.exp(L - mx[:, None]).sum(axis=1)
                  + (N1 - q1) * np.exp(-mx)) + mx

    pl = -(weights.astype(np.float64)
           * (mat_ene_sum.astype(np.float64) - logZ.astype(np.float64))).sum()

    sf2 = sf.reshape(H, -1).astype(np.float64)
    VV = V.reshape(H, -1).astype(np.float64)
    reg = LAMBD * ((sf2 @ sf2.T) * (VV @ VV.T)).sum()
    return np.array(pl + reg, dtype=np.float32)


def run_device(Tt, Eoh, trace=False, **kw):
    from concourse.bass_utils import run_bass_kernel_spmd

    global _PROGRAM
    if _PROGRAM is None:
        _PROGRAM = _build_program()
    in_maps = [
        {"Tt": Tt, "E": np.ascontiguousarray(Eoh[:, c * MS:(c + 1) * MS])}
        for c in range(NCORES)
    ]
    out = run_bass_kernel_spmd(_PROGRAM, in_maps, list(range(NCORES)),
                               trace=trace, **kw)
    Gf = np.concatenate([np.asarray(out.results[c]["G"]) for c in range(NCORES)],
                        axis=1)                          # (PDIM, M)
    return Gf, out


def kernel(**inputs):
    Q = np.asarray(inputs["Q"], np.float32)
    K = np.asarray(inputs["K"], np.float32)
    V = np.asarray(inputs["V"], np.float32)
    Z1 = np.asarray(inputs["Z1"])
    Z2 = np.asarray(inputs["Z2"])
    weights = np.asarray(inputs["weights"], np.float32)

    sf, Tt, Eoh = host_prep(Q, K, V, Z2)
    Gf, _ = run_device(Tt, Eoh)
    G = Gf.reshape(N1, q1, M).transpose(0, 2, 1)         # (N1, M, q1)
    return host_tail(G, sf, V, Z1, weights)



# revision 8
# speedup vs baseline: 348227.1756x; 348227.1756x over previous
"""Trainium2 kernel for nn_AttentionModel_PCA (embedding_lookup).

Math: with sf = softmax(Q^T K) per head,
  G[i,m,a] = sum_h sum_j sf[h,i,j] * V[h,a,Z2[j,m]]
           = sum_{(j,c)} T[(j,c),(i,a)] * E[(j,c),m]
where T[(j,c),(i,a)] = sum_h sf[h,i,j] V[h,a,c]  (tiny H=8 contraction)
and E is the one-hot expansion of Z2. The (5376 x 5376) @ (5376 x M)
GEMM producing G is the dominant cost and runs on the 8 NeuronCores
with M sharded (512 samples per core).

The GEMM runs in fp8e4 DoubleRow perf mode (0.5 PE cycles per output
row vs 4 for fp32): T is scaled by a power-of-2 s into fp8's sweet
spot and E's one-hot 1.0s are exact in fp8; PSUM accumulates fp32 and
the host divides G by s. Final tolerance is 2e-2; fp8 quantization of
T contributes ~0.2% after averaging over the 256-term sums.

Layout: contraction k = t2*256 + two*128 + kp (21 DoubleRow pairs of
128-partition tiles). Stationary weights for output tile pt are
Tp[pt] = [128 kp, 21 t2, 2 two, 128 op] fp8, host-packed so each
per-pt slab load is one contiguous 672KB DMA (5376B per partition).
E sits resident in SBUF as [128 kp, 21 t2, 2 two, 512 m] fp8.
"""

import sys

import numpy as np

for _p in ("/opt/trn_rl_repo",):
    if _p not in sys.path:
        sys.path.append(_p)

H, d, N1, N2, q1, q2, M = 8, 64, 256, 256, 21, 21, 4096
NCORES = 8
MS = M // NCORES          # 512 samples per core
KDIM = N2 * q2            # 5376 contraction (j,c)
PDIM = N1 * q1            # 5376 output rows (i,a)
KT2 = KDIM // 256         # 21 DoubleRow contraction pair-tiles
PT = PDIM // 128          # 42 output-row tiles
LAMBD = 0.001

_PROGRAM = None


def _build_program():
    """Raw bass pipeline: explicit standalone wait_ge + then_inc.

    sync  : 42 per-pt stationary slab loads (3-slot ring, gated by PE)
    scalar: E loaded in 21 per-pair chunks, then G stores
    tensor: 42*21 fp8 DoubleRow matmuls, 2 PSUM banks ping-pong by pt
    vector: PSUM -> SBUF copies (4-slot out ring, gated by stores)
    """
    import concourse.bass as bass
    import concourse.mybir as mybir

    nc = bass.Bass()
    f32 = mybir.dt.float32
    f8 = mybir.dt.float8e4
    Tp = nc.declare_dram_parameter("Tp", [PT, 128, KT2, 2, 128], f8,
                                   isOutput=False)
    E = nc.declare_dram_parameter("E", [KT2, 128, 2 * MS], f8, isOutput=False)
    G = nc.declare_dram_parameter("G", [PDIM, MS], f32, isOutput=True)

    NS = 3                   # slab ring slots
    NOT = 4                  # output ring slots

    with (
        nc.sbuf_tensor([128, KT2, 2, MS], f8) as E_sb,
        nc.sbuf_tensor([128, NS, KT2, 2, 128], f8) as slab,
        nc.sbuf_tensor([128, NOT, MS], f32) as ot,
        nc.psum_tensor([128, 2 * MS], f32) as acc,
        nc.semaphore("e_sem") as e_sem,
        nc.semaphore("dma_sem") as dma_sem,
        nc.semaphore("pe_cnt") as pe_cnt,      # pt fully accumulated by PE
        nc.semaphore("cp_sem") as cp_sem,      # PSUM->SBUF copies done
        nc.semaphore("st_sem") as st_sem,      # G stores done
        nc.Block() as block,
    ):

        @block.sync
        def _(sync):
            for pt in range(PT):
                if pt >= NS:
                    sync.wait_ge(pe_cnt, pt - NS + 1)
                sync.dma_start(slab[:, pt % NS], Tp[pt]).then_inc(dma_sem, 16)

        @block.tensor
        def _(tensor):
            for pt in range(PT):
                half = (pt % 2) * MS
                tensor.wait_ge(dma_sem, 16 * (pt + 1))
                if pt >= 2:
                    # bank reused from pt-2: wait for its copy
                    tensor.wait_ge(cp_sem, pt - 1)
                for t2 in range(KT2):
                    if pt == 0 and t2 == 0:
                        tensor.wait_ge(e_sem, 16)
                    mm = nc.tensor.matmul(
                        acc[:, half:half + MS],
                        slab[:, pt % NS, t2],
                        E_sb[:, t2],
                        start=(t2 == 0),
                        stop=(t2 == KT2 - 1),
                        perf_mode=mybir.MatmulPerfMode.DoubleRow,
                    )
                    if t2 == KT2 - 1:
                        mm.then_inc(pe_cnt, 1)

        @block.vector
        def _(vector):
            for pt in range(PT):
                half = (pt % 2) * MS
                vector.wait_ge(pe_cnt, pt + 1)
                if pt >= NOT:
                    vector.wait_ge(st_sem, 16 * (pt - NOT + 1))
                nc.vector.tensor_copy(
                    ot[:, pt % NOT], acc[:, half:half + MS]
                ).then_inc(cp_sem, 1)

        @block.scalar
        def _(scalar):
            scalar.dma_start(
                E_sb[:], E.rearrange("t p m -> p t m")
            ).then_inc(e_sem, 16)
            for pt in range(PT):
                scalar.wait_ge(cp_sem, pt + 1)
                scalar.dma_start(
                    G[pt * 128:(pt + 1) * 128, :], ot[:, pt % NOT]
                ).then_inc(st_sem, 16)

    return nc


def host_prep(Q, K, V, Z2):
    """softmax, packed fp8 T (scaled by power-of-2 s), one-hot E (fp8)."""
    import ml_dtypes

    f8 = ml_dtypes.float8_e4m3

    e = np.einsum("hdi,hdj->hij", Q, K, optimize=True)
    e -= e.max(axis=2, keepdims=True)
    np.exp(e, out=e)
    sf = e / e.sum(axis=2, keepdims=True)
    Tt = np.einsum("hij,hac->jcia", sf, V, optimize=True).reshape(KDIM, PDIM)
    Tt = np.ascontiguousarray(Tt, np.float32)

    s = float(2.0 ** np.floor(np.log2(120.0 / max(np.abs(Tt).max(), 1e-30))))
    # [k, p] -> [t2, two, kp, pt, op] -> [pt, kp, t2, two, op]
    Tp = (Tt * s).reshape(KT2, 2, 128, PT, 128).transpose(3, 2, 0, 1, 4)
    Tp = np.ascontiguousarray(Tp).astype(f8)

    Mloc = Z2.shape[1]
    Eoh = np.zeros((KDIM, Mloc), f8)
    rows = (np.arange(N2, dtype=np.int64)[:, None] * q2 + Z2.astype(np.int64))
    Eoh[rows, np.arange(Mloc, dtype=np.int64)[None, :]] = 1.0
    return sf, Tt, (Tp, s), Eoh


def host_tail(G, sf, V, Z1, weights):
    """take_along_axis + logsumexp + loss + regularizer on (N1, M, q1) G."""
    Z1i = Z1.astype(np.int64)
    mat_ene_sum = np.take_along_axis(G, Z1i[:, :, None], axis=2)[..., 0].sum(axis=0)

    Gm = G.max(axis=0)                                   # (M, q1)
    L = np.log(np.exp(G - Gm).sum(axis=0)) + Gm          # (M, q1)
    mx = np.maximum(L.max(axis=1), 0.0)
    logZ = np.log(np.exp(L - mx[:, None]).sum(axis=1)
                  + (N1 - q1) * np.exp(-mx)) + mx

    pl = -(weights.astype(np.float64)
           * (mat_ene_sum.astype(np.float64) - logZ.astype(np.float64))).sum()

    sf2 = sf.reshape(H, -1).astype(np.float64)
    VV = V.reshape(H, -1).astype(np.float64)
    reg = LAMBD * ((sf2 @ sf2.T) * (VV @ VV.T)).sum()
    return np.array(pl + reg, dtype=np.float32)


def run_device(Tp_s, Eoh, trace=False, **kw):
    from concourse.bass_utils import run_bass_kernel_spmd

    Tp, s = Tp_s
    global _PROGRAM
    if _PROGRAM is None:
        _PROGRAM = _build_program()
    # E chunk layout per core: [t2, kp, two*m]
    in_maps = []
    for c in range(NCORES):
        Ec = Eoh[:, c * MS:(c + 1) * MS]
        Ep = np.ascontiguousarray(
            Ec.reshape(KT2, 2, 128, MS).transpose(0, 2, 1, 3)
        ).reshape(KT2, 128, 2 * MS)
        in_maps.append({"Tp": Tp, "E": Ep})
    out = run_bass_kernel_spmd(_PROGRAM, in_maps, list(range(NCORES)),
                               trace=trace, **kw)
    Gf = np.concatenate([np.asarray(out.results[c]["G"]) for c in range(NCORES)],
                        axis=1).astype(np.float32)        # (PDIM, M)
    Gf /= s
    return Gf, out


def kernel(**inputs):
    Q = np.asarray(inputs["Q"], np.float32)
    K = np.asarray(inputs["K"], np.float32)
    V = np.asarray(inputs["V"], np.float32)
    Z1 = np.asarray(inputs["Z1"])
    Z2 = np.asarray(inputs["Z2"])
    weights = np.asarray(inputs["weights"], np.float32)

    sf, _Tt, Tp_s, Eoh = host_prep(Q, K, V, Z2)
    Gf, _ = run_device(Tp_s, Eoh)
    G = Gf.reshape(N1, q1, M).transpose(0, 2, 1)         # (N1, M, q1)
    return host_tail(G, sf, V, Z1, weights)
